# revision 13
# baseline (speedup 1.0000x reference)
"""Locally-connected 3x3 block (LCBlock) Trainium2 kernel.

Computes out = ELU(einsum('ocdkij,bcdkij->boij', weights, unfold(x)))
for x:[16,32,64,64] f32, weights:[32,32,3,3,64,64] f32.

Strategy (8 NeuronCores, SPMD, no collectives):
  - Spatially shard H=64 into 8 strips of 8 rows; each core gets its strip's
    per-position weights (they shard perfectly) and a 10-row halo'd slab of x.
  - Per position p=(y,x) the LC contraction is a tiny matmul
    [B=16, CK=288] x [CK=288, O=32].  We run it on the PE as 3 PSUM-accumulated
    matmuls (one per dj kernel column): lhsT = patch [K=96=(3di x 32c), M=16b]
    (cheap LDWEIGHTS: cost scales with columns=16), rhs = weights
    [96, 32o] (the big tensor streams as the moving operand).  4 positions run
    concurrently in the 4 PE column-groups via tile_position.
  - bf16 operands (fp32 PSUM accumulation) halve the HBM roofline.
  - ELU = max(x, exp(min(x,0))-1): 2 DVE ops + 1 ACT op per row-wave.
Host side packs/scatters inputs and gathers the 8 output strips.

The kernel is DMA-stream-bound (~15.6 GB/s x 16 SDMA engines per core under
full 8-core load), so the tuned default (BUILD_KW) minimizes streamed bytes
and keeps the 16 engines saturated end-to-end:
  - wsplit: each row-wave's weight DMA is issued as two halves on the two
    HWDGE queues (sync + scalar rings) -> 6 KB descriptors, both queues feed
    the shared engine pool concurrently.
  - xdedup: x is loaded once as a 10-row [32, 10560] slab (0.68 MB instead
    of a 3x di-replicated 1.62 MB) and replicated on-chip into the [96, .]
    patch layout via identity matmuls on the idle PE (PSUM -> SBUF bf16
    copies on DVE), saving ~1 MB of the ~12 MB per-core stream.
  - obf16: ELU result is written/DMA'd as bf16 (output traffic halved;
    rel-err ~4e-3, well inside the 2e-2 gate).
  - tailv2: output DMA goes out in chunks (rows 0..5 on the idle gpsimd ring
    as soon as their ELU lands, final rows 6..7 spread across 3 rings) so
    only a ~1 MB half-strip write trails the weight stream.
"""

import os
import sys

import numpy as np

for _p in ("/opt/trn_rl_repo", "/root/.axon_site/_ro/trn_rl_repo"):
    if os.path.isdir(_p) and _p not in sys.path:
        sys.path.insert(0, _p)

import ml_dtypes

import concourse.bacc as bacc
import concourse.mybir as mybir
import concourse.tile as tile
from concourse.bass_interp import get_hw_module
from concourse.bass_utils import run_bass_kernel_spmd

BF16 = ml_dtypes.bfloat16
E3M4 = ml_dtypes.float8_e3m4

# Problem shape (hardcoded per contract).
B, C, O, H, W = 16, 32, 32, 64, 64
NCORES = 8
HL = H // NCORES  # local rows per core
KW = 3  # conv kernel size
PART = KW * C  # 96 partitions: (di, c)
XW = W + 2  # padded row width
XFREE = HL * XW * B  # x slab free elems/partition
WCH = 4 * 16 * KW * O  # weight elems/partition per row-wave (j, pbl, dj, o)
WFREE = HL * WCH
OUTF = HL * 16 * O  # out free elems/partition: (w, pbl, o)

_CACHE = {}


def _hw2_layout():
    """Half-wave column layout: blocks (h, xx, x_lo, x_hi, coff) with the
    A half covering psum col-blocks x 0..7 and B covering 8..15; weight
    columns for the A half occupy [0, WCH/2), B [WCH/2, WCH)."""
    blocks = []
    coff = 0
    for h in range(2):
        xl, xh = (0, 7) if h == 0 else (8, 15)
        for xx in range(18):
            x_lo, x_hi = max(xl, xx - 2), min(xh, xx)
            if x_lo > x_hi:
                continue
            blocks.append((h, xx, x_lo, x_hi, coff))
            coff += 4 * (x_hi - x_lo + 1) * O
    assert coff == WCH, coff
    first_b = next(i for i, b in enumerate(blocks) if b[0] == 1)
    assert blocks[first_b][4] == WCH // 2, blocks[first_b]
    return blocks


_HW2_BLOCKS = _hw2_layout()


def _build(hw=True, reps=1, variant="full", loop_n=None, rpw=1, wbufs=None,
           x2=False, so2=False, ring2=False, walt=False, wsplit=False,
           obf16=False, tailv=False, esplit=0, gpm=False, tailv2=False,
           xdedup=False, hw2=False, tailv3=False, wsy=None):
    nc = bacc.Bacc(
        "TRN2", target_bir_lowering=False, debug=False, num_devices=NCORES
    )
    odt = mybir.dt.bfloat16 if obf16 else mybir.dt.float32
    X2F = 10 * XW * B  # dedup x slab: 10 unique halo'd rows
    if xdedup:
        xs2_d = nc.dram_tensor(
            "xs2", [C, X2F], mybir.dt.bfloat16, kind="ExternalInput"
        )
        id_d = nc.dram_tensor(
            "idm", [C, C], mybir.dt.bfloat16, kind="ExternalInput"
        )
    else:
        xs_d = nc.dram_tensor(
            "xs", [PART, XFREE], mybir.dt.bfloat16, kind="ExternalInput"
        )
    w_d = nc.dram_tensor("w", [PART, WFREE], mybir.dt.bfloat16, kind="ExternalInput")
    out_d = nc.dram_tensor("out", [4, 16, OUTF], odt, kind="ExternalOutput")

    if wbufs is None:
        wbufs = {1: 3, 2: 3, 4: 2, 8: 1}[rpw]
    with tile.TileContext(nc) as tc:
        with (
            tc.tile_pool(name="xp", bufs=1) as xp,
            tc.tile_pool(name="wp", bufs=wbufs) as wp,
            tc.tile_pool(name="pp", bufs=3, space="PSUM") as pp,
            tc.tile_pool(name="rp", bufs=2, space="PSUM") as rp,
            tc.tile_pool(name="op", bufs=1) as op,
            tc.tile_pool(name="tp", bufs=2) as tp,
        ):
          import contextlib

          loop_cm = tc.For_i(0, loop_n, 1) if loop_n else contextlib.nullcontext()
          with loop_cm:
           for _rep in range(reps):
            eng2 = nc.scalar if ring2 else nc.sync
            x_t = xp.tile([PART, XFREE], mybir.dt.bfloat16, tag="x")
            if xdedup:
                id_t = xp.tile([C, C], mybir.dt.bfloat16, tag="id")
                eng2.dma_start(id_t[:], id_d[:])
                x2_t = xp.tile([C, X2F], mybir.dt.bfloat16, tag="x2")
                eng2.dma_start(x2_t[:], xs2_d[:])
            elif x2:
                cut = 2 * XW * B
                eng2.dma_start(x_t[:, :cut], xs_d[:][:, :cut])
                eng2.dma_start(x_t[:, cut:], xs_d[:][:, cut:])
            else:
                eng2.dma_start(x_t[:], xs_d[:])
            out_t = op.tile([128, OUTF], odt, tag="o")

            repl_next = [0]

            def ensure_repl(col_limit):
                # on-chip replication x2[c, y+di] -> x[di*32+c, y] via
                # identity matmuls (PE partition broadcast), 512-col chunks
                while repl_next[0] * 512 < min(col_limit, XFREE):
                    ci = repl_next[0]
                    repl_next[0] += 1
                    c0 = ci * 512
                    cs = min(512, XFREE - c0)
                    ps2 = rp.tile([128, 512], mybir.dt.float32, tag="r")
                    for di in range(KW):
                        nc.tensor.matmul(
                            ps2[32 * di:32 * di + C, :cs],
                            id_t[:, :],
                            x2_t[:, di * XW * B + c0:di * XW * B + c0 + cs],
                            start=True,
                            stop=True,
                            skip_group_check=True,
                            tile_position=(0, 32 * di),
                        )
                    nc.vector.tensor_copy(x_t[:, c0:c0 + cs], ps2[:PART, :cs])

            for wg in range(HL // rpw):  # rpw image rows per DMA chunk
              if xdedup:
                  ensure_repl((wg * rpw + rpw + 1) * XW * B)
              w_t = wp.tile([PART, rpw * WCH], mybir.dt.bfloat16, tag="w")
              wlo = wg * rpw * WCH
              if wsplit == 3:
                  th = rpw * WCH // 3
                  nc.sync.dma_start(w_t[:, :th], w_d[:][:, wlo:wlo + th])
                  nc.scalar.dma_start(
                      w_t[:, th:2 * th], w_d[:][:, wlo + th:wlo + 2 * th]
                  )
                  nc.gpsimd.dma_start(
                      w_t[:, 2 * th:], w_d[:][:, wlo + 2 * th:wlo + 3 * th]
                  )
              elif wsplit:
                  # asymmetric split: sync also streams x2/x, so give it
                  # fewer weight columns and both queues drain together
                  h = wsy if wsy else rpw * WCH // 2
                  nc.sync.dma_start(w_t[:, :h], w_d[:][:, wlo:wlo + h])
                  nc.scalar.dma_start(
                      w_t[:, h:], w_d[:][:, wlo + h:wlo + rpw * WCH]
                  )
              else:
                  weng = (nc.sync, nc.scalar)[wg % 2] if walt else nc.sync
                  weng.dma_start(w_t[:], w_d[:][:, wlo:wlo + rpw * WCH])
              for r in range(rpw):
                wv = wg * rpw + r
                if hw2:
                    # half-wave granularity: A half (psum cols 0..255,
                    # weight cols [0,WCH/2) via sync) computes + ELUs while
                    # the B half's weights are still streaming -> pipeline
                    # drain after the last weight byte is half as deep
                    assert rpw == 1 and wsplit and variant == "full"

                    def _elu_h(pst, hh):
                        t1 = tp.tile([128, 256], mybir.dt.float32, tag="t1")
                        nc.vector.tensor_scalar_min(t1[:], pst[:], 0.0)
                        nc.scalar.activation(
                            t1[:], t1[:], mybir.ActivationFunctionType.Exp
                        )
                        nc.vector.scalar_tensor_tensor(
                            out_t[:, wv * 512 + 256 * hh:
                                  wv * 512 + 256 * (hh + 1)],
                            t1[:],
                            -1.0,
                            pst[:],
                            op0=mybir.AluOpType.add,
                            op1=mybir.AluOpType.max,
                        )

                    psh0 = pp.tile([128, 256], mybir.dt.float32, tag="ps0")
                    psh1 = pp.tile([128, 256], mybir.dt.float32, tag="ps1")
                    psh = [psh0, psh1]
                    nc.vector.memset(psh[0][:], 0.0)
                    nc.vector.memset(psh[1][:], 0.0)
                    done_a = False
                    for (h, xx, x_lo, x_hi, coff) in _HW2_BLOCKS:
                        if h == 1 and not done_a:
                            _elu_h(psh[0], 0)
                            done_a = True
                        n = x_hi - x_lo + 1
                        for j in range(4):
                            lo = (wv * XW + 16 * j + xx) * B
                            nc.tensor.matmul(
                                psh[h][32 * j:32 * j + B,
                                       32 * (x_lo - 8 * h):
                                       32 * (x_hi + 1 - 8 * h)],
                                x_t[:, lo:lo + B],
                                w_t[:, coff + j * n * O:
                                     coff + (j + 1) * n * O],
                                start=False,
                                stop=True,
                                skip_group_check=True,
                                tile_position=(0, 32 * j),
                            )
                    _elu_h(psh[1], 1)
                else:
                 ps = pp.tile([128, 512], mybir.dt.float32, tag="ps")
                 # zero-fill: matmuls pure-accumulate (start=False) onto
                 # this; ELU reads rows the col-tiled matmuls never touch
                 (nc.gpsimd if gpm else nc.vector).memset(ps[:], 0.0)
                 if variant != "dma_only":
                    # one MM per (xx, j): patch col xx serves dj=0,1,2 for
                    # positions x = xx, xx-1, xx-2 (adjacent PSUM slots)
                    coff = 0
                    for xx in range(18):
                        x_lo, x_hi = max(0, xx - 2), min(15, xx)
                        n = x_hi - x_lo + 1
                        for j in range(4):
                            lo = (wv * XW + 16 * j + xx) * B
                            nc.tensor.matmul(
                                ps[32 * j:32 * j + B,
                                   32 * x_lo:32 * (x_hi + 1)],
                                x_t[:, lo:lo + B],
                                w_t[:, r * WCH + coff + j * n * O:
                                     r * WCH + coff + (j + 1) * n * O],
                                start=False,
                                stop=True,
                                skip_group_check=True,
                                tile_position=(0, 32 * j),
                            )
                        coff += 4 * n * O
                 if variant in ("full",):
                    # ELU: out = max(psum, exp(min(psum, 0)) - 1)
                    # Last wave: column-chunked with min/exp/stt emission
                    # grouped by op so the in-order rings pipeline chunks
                    # (psum region deps let early chunks start before the
                    # wave's final matmuls land).
                    nch = esplit if (esplit and wv == HL - 1) else 1
                    t1 = tp.tile([128, 512], mybir.dt.float32, tag="t1")
                    bnds = [
                        (h * 512 // nch, (h + 1) * 512 // nch)
                        for h in range(nch)
                    ]
                    for cl, ch_ in bnds:
                        nc.vector.tensor_scalar_min(
                            t1[:, cl:ch_], ps[:, cl:ch_], 0.0
                        )
                    for cl, ch_ in bnds:
                        nc.scalar.activation(
                            t1[:, cl:ch_], t1[:, cl:ch_],
                            mybir.ActivationFunctionType.Exp,
                        )
                    for cl, ch_ in bnds:
                        nc.vector.scalar_tensor_tensor(
                            out_t[:, wv * 512 + cl:wv * 512 + ch_],
                            t1[:, cl:ch_],
                            -1.0,
                            ps[:, cl:ch_],
                            op0=mybir.AluOpType.add,
                            op1=mybir.AluOpType.max,
                        )
                 else:
                    # cheap evacuation so deps/out exist: copy psum -> out
                    nc.vector.tensor_copy(
                        out_t[:, wv * 512:(wv + 1) * 512], ps[:]
                    )
                if tailv3:
                    # bulk out chunk gated on wave HL-2's ELU so its packets
                    # land in the engine-idle window after the weight
                    # stream ends (instead of interleaving with its tail)
                    oap = out_d.ap()
                    if wv == HL - 2:
                        c0, c1 = 0, (HL - 1) * 512
                        for j in range(4):
                            nc.gpsimd.dma_start(
                                oap[j][:, c0:c1],
                                out_t[32 * j:32 * j + 16, c0:c1],
                            )
                    elif wv == HL - 1:
                        c0, c1 = (HL - 1) * 512, HL * 512
                        rings = [nc.sync, nc.scalar, nc.gpsimd, nc.sync]
                        for j in range(4):
                            rings[j].dma_start(
                                oap[j][:, c0:c1],
                                out_t[32 * j:32 * j + 16, c0:c1],
                            )
                elif tailv2:
                    # [0..HL-3] on idle gpsimd mid-stream; merged final
                    # [HL-2..HL-1] chunk spread over 3 rings at the end
                    oap = out_d.ap()
                    if wv == HL - 3:
                        c0, c1 = 0, (HL - 2) * 512
                        for j in range(4):
                            nc.gpsimd.dma_start(
                                oap[j][:, c0:c1],
                                out_t[32 * j:32 * j + 16, c0:c1],
                            )
                    elif wv == HL - 1:
                        c0, c1 = (HL - 2) * 512, HL * 512
                        rings = [nc.sync, nc.scalar, nc.gpsimd, nc.sync]
                        for j in range(4):
                            rings[j].dma_start(
                                oap[j][:, c0:c1],
                                out_t[32 * j:32 * j + 16, c0:c1],
                            )
                elif tailv:
                    # out DMA chunks [0..HL-3], [HL-2], [HL-1]; 4 rings
                    rings = [nc.gpsimd, nc.sync, nc.scalar, nc.gpsimd]
                    bounds = {HL - 3: (0, (HL - 2) * 512),
                              HL - 2: ((HL - 2) * 512, (HL - 1) * 512),
                              HL - 1: ((HL - 1) * 512, HL * 512)}
                    if wv in bounds:
                        c0, c1 = bounds[wv]
                        oap = out_d.ap()
                        for j in range(4):
                            rings[j].dma_start(
                                oap[j][:, c0:c1],
                                out_t[32 * j:32 * j + 16, c0:c1],
                            )
                elif so2 and wv == HL // 2 - 1:
                    oap = out_d.ap()
                    half = (HL // 2) * 512
                    for j in range(4):
                        eng2.dma_start(
                            oap[j][:, :half], out_t[32 * j:32 * j + 16, :half]
                        )
            if not tailv and not tailv2 and not tailv3:
                oap = out_d.ap()
                half = (HL // 2) * 512 if so2 else 0
                for j in range(4):
                    eng2.dma_start(
                        oap[j][:, half:], out_t[32 * j:32 * j + 16, half:]
                    )

    nc.compile()
    if hw:
        nc.m = get_hw_module(nc.m)
    return nc


WSCALE = 256.0  # weights are streamed as fp8e3(256*w); x slab carries 1/256

XSCALE3 = 2.0    # v3: x streams as fp8e3(2*x), w as fp8e3(256*w)
DESCALE3 = 1.0 / (XSCALE3 * WSCALE)  # undone via ACT scale before the ELU


def _build3(hw=True, wbufs=4, xchunks=4, esplit2=2, ofin=True,
            oprog=(1, 3, 5), memeng="vector"):
    """v3: all-fp8 streams, no on-chip replication, minimal PE work.

    v2 turned out PE-dispatch-bound: the identity-matmul x-replication and
    the K=1 zero-matmuls pushed the tensor engine past the (now fp8-halved)
    DMA stream.  v3 ships x pre-replicated from HBM as fp8e3(2*x) (0.81 MB
    vs 0.34 dedup'd -- still noise next to the 4.72 MB weight stream) so the
    PE runs nothing but the 576 LDW+MM pairs, and the PSUM zeroing goes back
    to a DVE memset.  The 1/512 product scale is undone by the ACT engine's
    fused scale while evacuating PSUM (t2 = ps/512), which also shortens the
    PSUM critical section.  x rides the gpsimd ring in 2-wave chunks so
    wave-0 compute starts ~2 us into the stream; out chunks ride gpsimd
    mid-stream and the final wave lands on the idle sync/scalar rings.
    """
    nc = bacc.Bacc(
        "TRN2", target_bir_lowering=False, debug=False, num_devices=NCORES
    )
    xs_d = nc.dram_tensor("xs", [PART, XFREE], mybir.dt.float8e3,
                          kind="ExternalInput")
    w_d = nc.dram_tensor("w", [PART, WFREE], mybir.dt.float8e3,
                         kind="ExternalInput")
    out_d = nc.dram_tensor("out", [4, 16, OUTF], mybir.dt.bfloat16,
                           kind="ExternalOutput")

    with tile.TileContext(nc) as tc:
        with (
            tc.tile_pool(name="xp", bufs=1) as xp,
            tc.tile_pool(name="wp", bufs=wbufs) as wp,
            tc.tile_pool(name="pp", bufs=3, space="PSUM") as pp,
            tc.tile_pool(name="op", bufs=1) as op,
            tc.tile_pool(name="tp", bufs=2) as tp,
            tc.tile_pool(name="sp", bufs=2) as sp,
        ):
            x_t = xp.tile([PART, XFREE], mybir.dt.float8e3, tag="x")
            per = HL // xchunks * XW * B
            # first two x chunks ride the HWDGE rings ahead of wave-0/1
            # weights (wave-0 compute starts ~2.4us into the stream); the
            # rest go to gpsimd, which is otherwise idle until out writes
            xrings = [nc.sync, nc.scalar] + [nc.gpsimd] * (xchunks - 2)
            for k in range(xchunks):
                c0 = k * per
                c1 = XFREE if k == xchunks - 1 else (k + 1) * per
                xrings[k].dma_start(x_t[:, c0:c1], xs_d[:][:, c0:c1])
            out_t = op.tile([128, OUTF], mybir.dt.bfloat16, tag="o")
            meng = getattr(nc, memeng)

            oap = out_d.ap()
            # [4, 16, OUTF] view of out_t (partitions 32j..32j+16 per j) so
            # each out chunk is ONE dma_start instead of four per-j ones
            out_g = out_t[:].rearrange("(g p) f -> g p f", g=4)[:, 0:16, :]
            for wv in range(HL):
                w_t = wp.tile([PART, WCH], mybir.dt.float8e3, tag="w")
                wlo = wv * WCH
                h = WCH // 2
                nc.sync.dma_start(w_t[:, :h], w_d[:][:, wlo:wlo + h])
                nc.scalar.dma_start(w_t[:, h:], w_d[:][:, wlo + h:wlo + WCH])

                ps = pp.tile([128, 512], mybir.dt.float32, tag="ps")
                meng.memset(ps[:], 0.0)
                coff = 0
                for xx in range(18):
                    x_lo, x_hi = max(0, xx - 2), min(15, xx)
                    n = x_hi - x_lo + 1
                    for j in range(4):
                        lo = (wv * XW + 16 * j + xx) * B
                        nc.tensor.matmul(
                            ps[32 * j:32 * j + B, 32 * x_lo:32 * (x_hi + 1)],
                            x_t[:, lo:lo + B],
                            w_t[:, coff + j * n * O:coff + (j + 1) * n * O],
                            start=False,
                            stop=True,
                            skip_group_check=True,
                            tile_position=(0, 32 * j),
                        )
                    coff += 4 * n * O
                # ELU with fused descale:
                #   t2 = ps/512 (ACT copy-scale, also evacuates PSUM)
                #   out = max(exp(min(t2,0)) - 1, t2)
                nch = esplit2 if (esplit2 and wv == HL - 1 and ofin) else 1
                t1 = tp.tile([128, 512], mybir.dt.float32, tag="t1")
                t2 = sp.tile([128, 512], mybir.dt.float32, tag="t2")
                bnds = [(q * 512 // nch, (q + 1) * 512 // nch)
                        for q in range(nch)]
                for ci, (cl, ch_) in enumerate(bnds):
                    nc.scalar.activation(
                        t2[:, cl:ch_], ps[:, cl:ch_],
                        mybir.ActivationFunctionType.Copy, scale=DESCALE3,
                    )
                    nc.vector.tensor_scalar_min(
                        t1[:, cl:ch_], t2[:, cl:ch_], 0.0
                    )
                    nc.scalar.activation(
                        t1[:, cl:ch_], t1[:, cl:ch_],
                        mybir.ActivationFunctionType.Exp,
                    )
                    nc.vector.scalar_tensor_tensor(
                        out_t[:, wv * 512 + cl:wv * 512 + ch_],
                        t1[:, cl:ch_],
                        -1.0,
                        t2[:, cl:ch_],
                        op0=mybir.AluOpType.add,
                        op1=mybir.AluOpType.max,
                    )
                    if wv == HL - 1 and ofin:
                        rings = [nc.sync, nc.scalar]
                        rings[ci % 2].dma_start(
                            oap[:, :, wv * 512 + cl:wv * 512 + ch_],
                            out_g[:, :, wv * 512 + cl:wv * 512 + ch_],
                        )
                if wv in oprog:
                    prev = [p for p in oprog if p < wv]
                    c0 = (max(prev) + 1) * 512 if prev else 0
                    c1 = (wv + 1) * 512
                    nc.gpsimd.dma_start(
                        oap[:, :, c0:c1], out_g[:, :, c0:c1]
                    )
            tail_lo = (max(oprog) + 1) * 512
            tail_hi = (HL - 1) * 512 if ofin else HL * 512
            if tail_hi > tail_lo:
                nc.gpsimd.dma_start(
                    oap[:, :, tail_lo:tail_hi], out_g[:, :, tail_lo:tail_hi]
                )

    nc.compile()
    if hw:
        nc.m = get_hw_module(nc.m)
    return nc


def _build2(hw=True, xfp8=False, wsy=None, zmm=True, rsc=True, esplit2=2,
            ofin=True, oprog=(1, 3, 5)):
    """v2 kernel: fp8e3 weights, gpsimd x-stream, PE psum-zeroing,
    progressive output DMA, chunked final wave.

    - Weights stream as fp8e3 of (256*w) [4.72 MB/core]; the 1/256 is folded
      into the x slab (bf16 of x/256) so PSUM holds true w*x sums.
    - xfp8: x slab additionally fp8e3 of (2*x) with the 1/512 folded into the
      replication identity matrix.
    - x2+id DMA ride the gpsimd (SWDGE) ring so both HWDGE rings stream
      weights from the first instruction; out chunks also ride gpsimd except
      the final wave, which lands on the then-idle sync/scalar rings.
    - zmm: per-wave PSUM zeroing via a K=1 zero-matmul on the PE (start=True
      clears has_written; real matmuls accumulate with start=False), freeing
      DVE; rsc: replication PSUM->SBUF copies on ACT for the same reason.
    - final wave: ELU in esplit2 column chunks, each DMA'd immediately.
    """
    nc = bacc.Bacc(
        "TRN2", target_bir_lowering=False, debug=False, num_devices=NCORES
    )
    X2F = 10 * XW * B
    xdt = mybir.dt.float8e3 if xfp8 else mybir.dt.bfloat16
    xs2_d = nc.dram_tensor("xs2", [C, X2F], xdt, kind="ExternalInput")
    id_d = nc.dram_tensor("idm", [C, C], mybir.dt.bfloat16, kind="ExternalInput")
    w_d = nc.dram_tensor(
        "w", [PART, WFREE], mybir.dt.float8e3, kind="ExternalInput"
    )
    out_d = nc.dram_tensor(
        "out", [4, 16, OUTF], mybir.dt.bfloat16, kind="ExternalOutput"
    )

    with tile.TileContext(nc) as tc:
        with (
            tc.tile_pool(name="xp", bufs=1) as xp,
            tc.tile_pool(name="wp", bufs=3) as wp,
            tc.tile_pool(name="pp", bufs=3, space="PSUM") as pp,
            tc.tile_pool(name="rp", bufs=2, space="PSUM") as rp,
            tc.tile_pool(name="op", bufs=1) as op,
            tc.tile_pool(name="tp", bufs=2) as tp,
        ):
            id_t = xp.tile([C, C], mybir.dt.bfloat16, tag="id")
            nc.gpsimd.dma_start(id_t[:], id_d[:])
            x2_t = xp.tile([C, X2F], xdt, tag="x2")
            nc.gpsimd.dma_start(x2_t[:], xs2_d[:])
            x_t = xp.tile([PART, XFREE], mybir.dt.bfloat16, tag="x")
            out_t = op.tile([128, OUTF], mybir.dt.bfloat16, tag="o")
            if zmm:
                z_t = xp.tile([1, 640], mybir.dt.bfloat16, tag="z")
                nc.vector.memset(z_t[:], 0.0)

            repl_next = [0]

            def ensure_repl(col_limit):
                # on-chip replication x2[c, y+di] -> x[di*32+c, y] via
                # identity matmuls (PE partition broadcast), 512-col chunks
                while repl_next[0] * 512 < min(col_limit, XFREE):
                    ci = repl_next[0]
                    repl_next[0] += 1
                    c0 = ci * 512
                    cs = min(512, XFREE - c0)
                    ps2 = rp.tile([128, 512], mybir.dt.float32, tag="r")
                    for di in range(KW):
                        nc.tensor.matmul(
                            ps2[32 * di:32 * di + C, :cs],
                            id_t[:, :],
                            x2_t[:, di * XW * B + c0:di * XW * B + c0 + cs],
                            start=True,
                            stop=True,
                            skip_group_check=True,
                            tile_position=(0, 32 * di),
                        )
                    if rsc:
                        nc.scalar.activation(
                            x_t[:, c0:c0 + cs], ps2[:PART, :cs],
                            mybir.ActivationFunctionType.Copy,
                        )
                    else:
                        nc.vector.tensor_copy(x_t[:, c0:c0 + cs], ps2[:PART, :cs])

            oap = out_d.ap()
            for wv in range(HL):
                ensure_repl((wv + 2) * XW * B)
                w_t = wp.tile([PART, WCH], mybir.dt.float8e3, tag="w")
                wlo = wv * WCH
                h = wsy if wsy else WCH // 2
                nc.sync.dma_start(w_t[:, :h], w_d[:][:, wlo:wlo + h])
                nc.scalar.dma_start(w_t[:, h:], w_d[:][:, wlo + h:wlo + WCH])

                ps = pp.tile([128, 512], mybir.dt.float32, tag="ps")
                if zmm:
                    # K=1 zero-matmul: writes 0 everywhere, start=True clears
                    # has_written so the accumulating matmuls below add onto 0
                    nc.tensor.matmul(
                        ps[:, :], z_t[:1, :128], z_t[:1, 128:640],
                        start=True, stop=True, skip_group_check=True,
                    )
                else:
                    nc.vector.memset(ps[:], 0.0)
                # one MM per (xx, j): patch col xx serves dj=0,1,2 for
                # positions x = xx, xx-1, xx-2 (adjacent PSUM slots)
                coff = 0
                for xx in range(18):
                    x_lo, x_hi = max(0, xx - 2), min(15, xx)
                    n = x_hi - x_lo + 1
                    for j in range(4):
                        lo = (wv * XW + 16 * j + xx) * B
                        nc.tensor.matmul(
                            ps[32 * j:32 * j + B, 32 * x_lo:32 * (x_hi + 1)],
                            x_t[:, lo:lo + B],
                            w_t[:, coff + j * n * O:coff + (j + 1) * n * O],
                            start=False,
                            stop=True,
                            skip_group_check=True,
                            tile_position=(0, 32 * j),
                        )
                    coff += 4 * n * O
                # ELU: out = max(psum, exp(min(psum, 0)) - 1)
                nch = esplit2 if (esplit2 and wv == HL - 1 and ofin) else 1
                t1 = tp.tile([128, 512], mybir.dt.float32, tag="t1")
                bnds = [(q * 512 // nch, (q + 1) * 512 // nch)
                        for q in range(nch)]
                for ci, (cl, ch_) in enumerate(bnds):
                    nc.vector.tensor_scalar_min(
                        t1[:, cl:ch_], ps[:, cl:ch_], 0.0
                    )
                    nc.scalar.activation(
                        t1[:, cl:ch_], t1[:, cl:ch_],
                        mybir.ActivationFunctionType.Exp,
                    )
                    nc.vector.scalar_tensor_tensor(
                        out_t[:, wv * 512 + cl:wv * 512 + ch_],
                        t1[:, cl:ch_],
                        -1.0,
                        ps[:, cl:ch_],
                        op0=mybir.AluOpType.add,
                        op1=mybir.AluOpType.max,
                    )
                    if wv == HL - 1 and ofin:
                        # final wave: land each chunk on the now-idle
                        # HWDGE rings the moment its ELU is done
                        rings = [nc.sync, nc.scalar]
                        for j in range(4):
                            rings[(ci + j) % 2].dma_start(
                                oap[j][:, wv * 512 + cl:wv * 512 + ch_],
                                out_t[32 * j:32 * j + 16,
                                      wv * 512 + cl:wv * 512 + ch_],
                            )
                # progressive bulk out on the gpsimd ring mid-stream
                if wv in oprog:
                    prev = [p for p in oprog if p < wv]
                    c0 = (max(prev) + 1) * 512 if prev else 0
                    c1 = (wv + 1) * 512
                    for j in range(4):
                        nc.gpsimd.dma_start(
                            oap[j][:, c0:c1], out_t[32 * j:32 * j + 16, c0:c1]
                        )
            # anything not yet flushed (waves after last oprog, minus the
            # final wave when ofin handles it)
            tail_lo = (max(oprog) + 1) * 512
            tail_hi = (HL - 1) * 512 if ofin else HL * 512
            if tail_hi > tail_lo:
                for j in range(4):
                    nc.gpsimd.dma_start(
                        oap[j][:, tail_lo:tail_hi],
                        out_t[32 * j:32 * j + 16, tail_lo:tail_hi],
                    )

    nc.compile()
    if hw:
        nc.m = get_hw_module(nc.m)
    return nc


def _pack_inputs(x, weights, xdedup=False, hw2=False, wfp8=False, xfp8=False):
    """Host-side scatter: per-core slabs (bf16, or fp8e3 for v2 weights)."""
    xpad32 = np.pad(x, ((0, 0), (0, 0), (1, 1), (1, 1))).astype(np.float32)
    if wfp8:
        # weights stream as fp8e3 of (256*w); fold 1/256 into the x slab
        # (and additionally 1/512 into the identity when the x slab is fp8e3
        # of 2*x) so PSUM holds the true w*x contraction.
        if xfp8:
            xpad = (2.0 * xpad32).astype(E3M4)
            idm = (np.eye(C, dtype=np.float32) / 512.0).astype(BF16)
        else:
            xpad = (xpad32 / WSCALE).astype(BF16)
            idm = np.eye(C, dtype=BF16)
        wb = np.asarray(weights, dtype=np.float32) * WSCALE  # cast per-block
        wdt = E3M4
    else:
        xpad = xpad32.astype(BF16)
        wb = np.asarray(weights).astype(BF16)  # [O,C,3,3,H,W]
        idm = np.eye(C, dtype=BF16)
        wdt = BF16
    in_maps = []
    for k in range(NCORES):
        if xdedup:
            # dedup slab: [c, y', xx, b] = xpad[b, c, 8k+y', xx], y' in 0..9
            xs2_k = np.ascontiguousarray(
                np.transpose(xpad[:, :, 8 * k:8 * k + HL + 2, :], (1, 2, 3, 0))
            ).reshape(C, (HL + 2) * XW * B)
        else:
            # x slab: [di*32+c, y, xx, b] = xpad[b, c, 8k+y+di, xx]
            slabs = [
                np.transpose(
                    xpad[:, :, 8 * k + di:8 * k + di + HL, :], (1, 2, 3, 0)
                )
                for di in range(KW)
            ]
            xs_k = np.ascontiguousarray(np.stack(slabs, 0)).reshape(PART, XFREE)
        # weights, merged-xx layout: per (y, xx, j), 32-col blocks for
        # x = x_lo..x_hi ascending (dj = xx-x descending):
        #   block = W[o, c, di, dj, 8k+y, 16j+x] as [di*32+c, y, o]
        wc = np.transpose(
            wb[:, :, :, :, 8 * k:8 * (k + 1), :], (2, 1, 3, 4, 5, 0)
        )  # [di, c, dj, y, x, o]
        wc = wc.reshape(PART, KW, HL, W, O)  # [(di,c), dj, y, x, o]
        w_k = np.empty((PART, HL, WCH), dtype=wdt)
        if hw2:
            blocks = _HW2_BLOCKS
        else:
            blocks, coff = [], 0
            for xx in range(18):
                x_lo, x_hi = max(0, xx - 2), min(15, xx)
                blocks.append((0, xx, x_lo, x_hi, coff))
                coff += 4 * (x_hi - x_lo + 1) * O
        for (_h, xx, x_lo, x_hi, coff) in blocks:
            n = x_hi - x_lo + 1
            for j in range(4):
                for t, xr in enumerate(range(x_lo, x_hi + 1)):
                    dj = xx - xr
                    c0 = coff + j * n * O + t * O
                    # [(di,c), y, o]
                    w_k[:, :, c0:c0 + O] = wc[:, dj, :, 16 * j + xr, :]
        wf = w_k.reshape(PART, WFREE)
        if xdedup:
            in_maps.append({"xs2": xs2_k, "idm": idm, "w": wf})
        else:
            in_maps.append({"xs": xs_k, "w": wf})
    return in_maps


def _unpack_outputs(results):
    out = np.empty((B, O, H, W), dtype=np.float32)
    for k in range(NCORES):
        arr = results[k]["out"].reshape(4, 16, HL, 16, O)  # [j, b, w, slot, o]
        strip = np.transpose(arr, (1, 4, 2, 0, 3)).reshape(B, O, HL, W)
        out[:, :, 8 * k:8 * (k + 1), :] = strip
    return out


def _pack_inputs_v3(x, weights):
    """v3 host scatter: x slabs fp8e3(2*x) pre-replicated, w fp8e3(256*w)."""
    xpad = (XSCALE3 * np.pad(x, ((0, 0), (0, 0), (1, 1), (1, 1)))
            ).astype(np.float32)
    wb = np.asarray(weights, dtype=np.float32) * WSCALE
    in_maps = []
    for k in range(NCORES):
        # x slab: [di*32+c, y, xx, b] = 2*xpad[b, c, 8k+y+di, xx]
        slabs = [
            np.transpose(
                xpad[:, :, 8 * k + di:8 * k + di + HL, :], (1, 2, 3, 0)
            )
            for di in range(KW)
        ]
        xs_k = np.ascontiguousarray(np.stack(slabs, 0)).reshape(
            PART, XFREE).astype(E3M4)
        wc = np.transpose(
            wb[:, :, :, :, 8 * k:8 * (k + 1), :], (2, 1, 3, 4, 5, 0)
        )  # [di, c, dj, y, x, o]
        wc = wc.reshape(PART, KW, HL, W, O)
        w_k = np.empty((PART, HL, WCH), dtype=E3M4)
        coff = 0
        for xx in range(18):
            x_lo, x_hi = max(0, xx - 2), min(15, xx)
            n = x_hi - x_lo + 1
            for j in range(4):
                for t, xr in enumerate(range(x_lo, x_hi + 1)):
                    dj = xx - xr
                    c0 = coff + j * n * O + t * O
                    w_k[:, :, c0:c0 + O] = wc[:, dj, :, 16 * j + xr, :]
            coff += 4 * n * O
        in_maps.append({"xs": xs_k, "w": w_k.reshape(PART, WFREE)})
    return in_maps


# v1 tuned config: weights split across sync+scalar DMA queues,
# bf16 output, chunked/ring-spread output DMA, on-chip x replication.
BUILD_KW_V1 = dict(wsplit=True, obf16=True, tailv2=True, xdedup=True)
# v2 config: fp8e3 weights + on-chip replication (PE-dispatch-bound).
BUILD_KW_V2 = dict(v2=True)
# v3 default: all-fp8 streams, no replication (see _build3).
BUILD_KW = dict(v3=True)


def run(x, weights, trace=False, **build_kw):
    kw = dict(BUILD_KW, **build_kw)
    v2 = kw.pop("v2", False)
    v3 = kw.pop("v3", False)
    ver = "v3" if v3 else "v2" if v2 else "v1"
    key = (ver,) + tuple(sorted(kw.items()))
    if key not in _CACHE:
        bld = _build3 if v3 else _build2 if v2 else _build
        _CACHE[key] = bld(**kw)
    nc = _CACHE[key]
    if v3:
        in_maps = _pack_inputs_v3(np.asarray(x), np.asarray(weights))
    elif v2:
        in_maps = _pack_inputs(
            np.asarray(x), np.asarray(weights), xdedup=True,
            wfp8=True, xfp8=kw.get("xfp8", False),
        )
    else:
        in_maps = _pack_inputs(
            np.asarray(x), np.asarray(weights),
            xdedup=kw.get("xdedup", False), hw2=kw.get("hw2", False),
        )
    res = run_bass_kernel_spmd(nc, in_maps, list(range(NCORES)), trace=trace)
    return _unpack_outputs(res.results), res


def kernel(x, weights):
    out, _ = run(x, weights)
    return out

